# revision 74
# baseline (speedup 1.0000x reference)
"""Trainium2 Bass kernel for nn_DecoderHeadVDP (variance-propagating decoder
attention head), distributed over 8 NeuronCores.

Sharding: core c handles batch b = c//2 and head-group g = c%2 (8 of 16 heads,
i.e. output columns [512*g, 512*(g+1)) of the Wq projection).  Inputs are
pre-sliced, pre-transposed and pre-quantized on the host so all 8 cores run
one identical NEFF (true SPMD).

Math (per core, per head h):
  q      = x Wq_mu^T                      (score scale 1/32 folded into exp)
  var_q  = var_x W1^T + x^2 Wq_var^T        (W1 = Wq_var + Wq_mu^2)
  aT     = k qT / 32  (scores transposed [j, i]), causal (i >= j)
  sv'    = 2*(kv2 var_q + var_k q^2)        (true var_a = sv' / 2^11 / ...)
  p = exp(aT) masked; u = p^2; w' = u*sv'; pw' = p*w'
  AV stage SWAPPED: stationary = p/u/w'/pw' [j, i-block], moving = R [j, :]:
    out[i, 0:65]    = sum_j p  [v | 1]            -> M | Z
    out[i, 65:193]  = sum_j u  [var_v | vv2]      -> A3 | A1
    out[i, 193:258] = sum_j w' [vv2/2^11 | 2^-11] -> A2 | S'
    out[i, 258:322] = sum_j pw'[vv2/2^11]         -> A4
  out_mean = M/Z + x
  out_var  = (S'/Z^4) A1 + (1/Z^2)(A2 + A3) - (2/Z^3) A4
The swapped AV orientation lands everything [i-partition, col], so Z and S'
are per-partition scalars and the combine needs no transposes at all.

Performance notes:
 - phase A and both score matmuls run as fp8e4 DoubleRow (0.5 PE cycles per
   output row, 256-deep contraction per instruction); numerics validated
   offline (rel err ~6e-3 vs 2e-2 budget)
 - scv pairs its two variance terms into ONE DoubleRow op via the kvT plane
   layout (kT | 0 | kv2 | var_k) against Q4 planes (q | var_q | q^2)
 - score-space elementwise stays bf16: exp on ACT, sv' evac on Pool, u/w/pw
   muls in DVE 4x mode; all fp8 quantization scales folded into host-side
   tensors, ACT evac scales, the exp scale, and two extra scaled-R columns
"""

import sys

import numpy as np

if "/opt/trn_rl_repo" not in sys.path:
    sys.path.insert(0, "/opt/trn_rl_repo")

B, S, D, H = 4, 1024, 1024, 16
DH = 64          # head dim
P = 128          # partitions
NHC = 8          # heads per core
CT = 4           # head-pair tiles per core (q cols 4 * 128 = 512)
KD = 8           # contraction d-tiles (8 * 128 = 1024)
GCOL = 512       # output columns per core
RC = 258         # R tile columns: v|1|var_v|vv2|vv2*2^-11|2^-11

_BUILT = None


def _emit(nc, tc):
    import concourse.bass as bass  # noqa: F401
    import concourse.mybir as mybir

    f32 = mybir.dt.float32
    bf16 = mybir.dt.bfloat16
    f8 = mybir.dt.float8e4
    AF = mybir.ActivationFunctionType
    OP = mybir.AluOpType
    DR = mybir.MatmulPerfMode.DoubleRow

    xT_d = nc.dram_tensor("xT8", [P, KD, S], f8, kind="ExternalInput")
    varxT_d = nc.dram_tensor("varxT8", [P, KD, S], f8, kind="ExternalInput")
    wqmuT_d = nc.dram_tensor("wqmuT8", [P, KD, CT, P], f8, kind="ExternalInput")
    w1T_d = nc.dram_tensor("w1T8", [P, KD, CT, P], f8, kind="ExternalInput")
    wqvarT_d = nc.dram_tensor("wqvarT8", [P, KD, CT, P], f8,
                              kind="ExternalInput")
    kvT_d = nc.dram_tensor("kvT8", [NHC, P, 4, S], f8, kind="ExternalInput")
    R_d = nc.dram_tensor("Rh", [NHC, P, KD, RC], bf16, kind="ExternalInput")
    outm_d = nc.dram_tensor("out_mean", [S, GCOL], f32, kind="ExternalOutput")
    outv_d = nc.dram_tensor("out_var", [S, GCOL], f32, kind="ExternalOutput")

    # persistent q-side tensor: planes (q, var_q*2^3, q^2*2^-2), all fp8
    qpool = tc.alloc_tile_pool(name="qpool", bufs=1)
    Q4 = qpool.tile([P, 3, CT, S], f8)

    # ---------------- phase A: DMAs + Q projection --------------------------
    # per-dp-pair tiles so the first matmuls start after ~1/4 of the DMA bytes
    wqt_pool = tc.alloc_tile_pool(name="wqt", bufs=1)
    xt_pool = tc.alloc_tile_pool(name="xtp", bufs=1)
    Wmu, Xt, W1t, Vxt, X2t, Wvt = [], [], [], [], [], []
    for dp in range(4):
        dsl = slice(2 * dp, 2 * dp + 2)
        wm = wqt_pool.tile([P, 2, CT, P], f8, name=f"wm{dp}")
        xt = xt_pool.tile([P, 2, S], f8, name=f"xt{dp}")
        nc.sync.dma_start(wm, wqmuT_d[:, dsl])
        nc.sync.dma_start(xt, xT_d[:, dsl])
        Wmu.append(wm)
        Xt.append(xt)
    for dp in range(4):
        dsl = slice(2 * dp, 2 * dp + 2)
        w1 = wqt_pool.tile([P, 2, CT, P], f8, name=f"w1{dp}")
        vx = xt_pool.tile([P, 2, S], f8, name=f"vx{dp}")
        nc.sync.dma_start(w1, w1T_d[:, dsl])
        nc.sync.dma_start(vx, varxT_d[:, dsl])
        W1t.append(w1)
        Vxt.append(vx)
    for dp in range(4):
        dsl = slice(2 * dp, 2 * dp + 2)
        wv = wqt_pool.tile([P, 2, CT, P], f8, name=f"wv{dp}")
        nc.sync.dma_start(wv, wqvarT_d[:, dsl])
        Wvt.append(wv)
    for dp in range(4):
        # x^2 on the phase-A-idle DVE instead of a fourth 1MB DMA stream
        # (the 2^-2 scale is folded into the Wq_var weights: 2^13 -> 2^11)
        x2 = xt_pool.tile([P, 2, S], f8, name=f"x2{dp}")
        nc.vector.tensor_mul(x2, Xt[dp], Xt[dp])
        X2t.append(x2)

    psumQ = tc.alloc_tile_pool(name="psumQ", bufs=2, space="PSUM")
    for sc_i in range(2):
        ssl = slice(sc_i * 512, (sc_i + 1) * 512)
        for cp in range(2):  # ct pairs, so ACT evacuations batch 2-wide
            cts = (2 * cp, 2 * cp + 1)
            mps = psumQ.tile([P, 2, 512], f32, tag="mps",
                             name=f"mps{cp}_{sc_i}")
            for q, ct in enumerate(cts):
                for dp in range(4):
                    nc.tensor.matmul(
                        mps[:, q], Wmu[dp][:, :, ct], Xt[dp][:, :, ssl],
                        start=(dp == 0), stop=(dp == 3), perf_mode=DR,
                    )
            # q (fp8, natural scale); q^2 on the phase-A-idle DVE from the
            # fp8 q (the matching 2^-2 is folded into kvT's var_k plane)
            nc.scalar.activation(Q4[:, 0, 2 * cp:2 * cp + 2, ssl], mps,
                                 AF.Identity, scale=2.0 ** -5)
            nc.vector.tensor_mul(Q4[:, 2, 2 * cp:2 * cp + 2, ssl],
                                 Q4[:, 0, 2 * cp:2 * cp + 2, ssl],
                                 Q4[:, 0, 2 * cp:2 * cp + 2, ssl])
        for cp in range(2):
            cts = (2 * cp, 2 * cp + 1)
            vps = psumQ.tile([P, 2, 512], f32, tag="vps",
                             name=f"vps{cp}_{sc_i}")
            for q, ct in enumerate(cts):
                for dp in range(4):
                    nc.tensor.matmul(
                        vps[:, q], W1t[dp][:, :, ct], Vxt[dp][:, :, ssl],
                        start=(dp == 0), stop=False, perf_mode=DR,
                    )
                for dp in range(4):
                    nc.tensor.matmul(
                        vps[:, q], Wvt[dp][:, :, ct], X2t[dp][:, :, ssl],
                        start=False, stop=(dp == 3), perf_mode=DR,
                    )
            # var_q * 2^3  (PSUM holds var_q * 2^11)
            nc.scalar.activation(Q4[:, 1, 2 * cp:2 * cp + 2, ssl], vps,
                                 AF.Identity, scale=2.0 ** -8)
    psumQ.release()
    xt_pool.release()
    wqt_pool.release()

    # ---------------- phase B: per-head attention ---------------------------
    from concourse.masks import make_identity

    pairp = tc.alloc_tile_pool(name="pairp", bufs=3)
    sbB = tc.alloc_tile_pool(name="sbB", bufs=3)
    psumB = tc.alloc_tile_pool(name="psumB", bufs=1, space="PSUM")
    psumS = tc.alloc_tile_pool(name="psumS", bufs=2, space="PSUM")

    # causal-mask constants: one extra PE matmul accumulates -3000 into the
    # above-diagonal scores (exp then underflows to exactly 0), which keeps
    # the diagonal masking off the ACT->Pool->DVE critical chain
    cpool = tc.alloc_tile_pool(name="cmask", bufs=1)
    identb = cpool.tile([P, P], bf16)
    make_identity(nc, identb)
    tneg = cpool.tile([P, P], bf16)
    nc.gpsimd.memset(tneg, 0.0)
    nc.gpsimd.affine_select(
        out=tneg, in_=tneg, compare_op=mybir.AluOpType.is_ge,
        fill=-3000.0, base=0, pattern=[[1, P]], channel_multiplier=-1,
    )

    def prep_pair(t):
        # kvT planes: (kT*2^-2 | zeros | kv2*2^-2 | var_k*2^3), head's 64 dims
        # on its q-partition rows, zeros elsewhere (full-128 contraction)
        KVTs, Rs = [], []
        for r in range(2):
            KVTr = pairp.tile([P, 4, S], f8, tag=f"KVT{r}", name=f"KVT{r}_{t}")
            if t < 3:
                # plane 1 is the DR zero slot: memset once per pool buffer
                # (bufs=3, buffers are stable across rotations) instead of
                # DMAing 1MB of zeros over the kernel
                nc.gpsimd.memset(KVTr[:, 1:2], 0.0)
            nc.sync.dma_start(KVTr[:, 0:1], kvT_d[2 * t + r, :, 0:1])
            nc.sync.dma_start(KVTr[:, 2:4], kvT_d[2 * t + r, :, 2:4])
            KVTs.append(KVTr)
        for r in range(2):
            Rr = pairp.tile([P, KD, RC], bf16, tag=f"R{r}", name=f"R{r}_{t}")
            nc.sync.dma_start(Rr, R_d[2 * t + r])
            Rs.append(Rr)
        return KVTs, Rs

    def av_tj_emitters(h, ic, R, pv):
        """Per-tj AV matmul emitter closures for one (h, ic) unit (so they
        can be interleaved between the next unit's score steps).

        PSUM layout (3 banks total, leaving scv a third buffer):
          avtA [P, 4, 65]  : M | Z               (one bank, 1040B/part)
          avtB [P, 4, 256] : A1 | X | S' | A4   (193 cols used, ib-stride
                             1024B so every matmul out stays in one bank)
        X accumulates BOTH A3 (u x var_v) and A2 (w' x vv2s) in-place.
        """
        ntj = 4 * (ic + 1)
        uid = f"{h}_{ic}"
        avtA = psumB.tile([P, 4, 65], f32, tag="avtA", name=f"avtA{uid}")
        avtB = psumB.tile([P, 4, 256], f32, tag="avtB", name=f"avtB{uid}")

        def emit_tj(tj):
            # start=True zeroes the WHOLE 2KB PSUM bank (HW-verified), so
            # exactly ONE matmul per bank per unit carries start=True (the
            # first to touch it); every other stream accumulates onto the
            # bank's pending-zero bytes.
            p2, u2, w2, pw2, q, i0 = pv[tj]
            first = tj == 0
            for ib in range(4):
                if 4 * ic + ib < tj:
                    continue
                last = tj == 4 * ic + ib
                cs = slice(ic * 512 + ib * P - i0,
                           ic * 512 + ib * P - i0 + P)
                nc.tensor.matmul(
                    avtA[:, ib, 0:65], p2[:, q, cs], R[:, tj, 0:65],
                    start=first and ib == 0, stop=last,
                    skip_group_check=True,
                )
                nc.tensor.matmul(
                    avtB[:, ib, 64:129], w2[:, q, cs], R[:, tj, 193:258],
                    start=first and ib % 2 == 0, stop=last,
                    skip_group_check=True,
                )
                nc.tensor.matmul(
                    avtB[:, ib, 0:128], u2[:, q, cs], R[:, tj, 65:193],
                    start=False, stop=last, skip_group_check=True,
                )
                nc.tensor.matmul(
                    avtB[:, ib, 129:193], pw2[:, q, cs], R[:, tj, 193:257],
                    start=False, stop=last, skip_group_check=True,
                )

        return (avtA, avtB), [lambda tj=tj: emit_tj(tj) for tj in range(ntj)]

    def combine_direct(h, ic, avt):
        """Tail-latency combine for the FINAL unit: read the avt PSUM banks
        directly on DVE (no ACT evacuation, no Pool hops) - the banks are
        never needed again, and the shorter serial chain trims the drain."""
        avtA, avtB = avt
        uid = f"{h}_{ic}d"
        bsh = (P, 4, DH)
        zrp = sbB.tile([P, 4, 4], f32, tag="zrp", name=f"zrp{uid}")
        nc.vector.reciprocal(zrp[:, :, 0:1], avtA[:, :, 64:65])
        nc.vector.tensor_mul(zrp[:, :, 1:2], zrp[:, :, 0:1], zrp[:, :, 0:1])
        nc.vector.tensor_scalar_mul(zrp[:, :, 2:3], zrp[:, :, 0:1], -2.0)
        nc.vector.tensor_mul(zrp[:, :, 3:4], zrp[:, :, 1:2],
                             avtB[:, :, 128:129])
        om = sbB.tile([P, 4, DH], f32, tag="om", name=f"om{uid}")
        nc.vector.tensor_mul(om, avtA[:, :, 0:64],
                             zrp[:, :, 0:1].broadcast_to(bsh))
        t0 = sbB.tile([P, 4, DH], f32, tag="t0", name=f"t0{uid}")
        nc.vector.tensor_mul(t0, avtB[:, :, 129:193],
                             zrp[:, :, 2:3].broadcast_to(bsh))
        t1 = sbB.tile([P, 4, DH], f32, tag="t1", name=f"t1{uid}")
        nc.vector.tensor_mul(t1, avtB[:, :, 0:64],
                             zrp[:, :, 3:4].broadcast_to(bsh))
        ov = sbB.tile([P, 4, DH], f32, tag="ov", name=f"ov{uid}")
        nc.vector.tensor_add(t0, avtB[:, :, 64:128], t0)
        nc.vector.tensor_add(t0, t0, t1)
        nc.vector.tensor_mul(ov, t0, zrp[:, :, 1:2].broadcast_to(bsh))
        nc.sync.dma_start(
            outm_d[ic * 512:(ic + 1) * 512, h * 64:(h + 1) * 64]
            .rearrange("(ib p) d -> p ib d", p=P),
            om,
        )
        nc.sync.dma_start(
            outv_d[ic * 512:(ic + 1) * 512, h * 64:(h + 1) * 64]
            .rearrange("(ib p) d -> p ib d", p=P),
            ov,
        )

    def combine(h, ic, avt):
        """Output combine for one (h, ic) unit. ACT evacuates the avt PSUM
        banks to SBUF (Pool cannot read PSUM on hardware); the arithmetic
        then runs on Pool from SBUF, with the tiny Zr-power chain on DVE.

        out_var = Zr^2*( X - 2Zr*A4 + Zr^2*S'*A1 )
        ev cols: M(0:64) | Z(64:65) | A1(65:129) | X(129:193) |
                 S'(193:194) | A4(194:258)
        zrp cols: 0=Zr, 1=Zr^2, 2=-2Zr, 3=Zr^2*S'
        """
        avtA, avtB = avt
        uid = f"{h}_{ic}"
        bsh = (P, 4, DH)
        ev = sbB.tile([P, 4, 258], f32, tag="ev", bufs=2, name=f"ev{uid}")
        nc.scalar.activation(ev[:, :, 0:65], avtA, AF.Identity)
        nc.scalar.activation(ev[:, :, 65:258], avtB[:, :, 0:193],
                             AF.Identity)
        zrp = sbB.tile([P, 4, 4], f32, tag="zrp", name=f"zrp{uid}")
        nc.vector.reciprocal(zrp[:, :, 0:1], ev[:, :, 64:65])
        nc.gpsimd.tensor_mul(zrp[:, :, 1:2], zrp[:, :, 0:1], zrp[:, :, 0:1])
        nc.gpsimd.tensor_scalar_mul(zrp[:, :, 2:3], zrp[:, :, 0:1], -2.0)
        nc.gpsimd.tensor_mul(zrp[:, :, 3:4], zrp[:, :, 1:2],
                             ev[:, :, 193:194])
        om = sbB.tile([P, 4, DH], f32, tag="om", name=f"om{uid}")
        # mean = M * Zr  (the +x residual is added on the host)
        nc.gpsimd.tensor_mul(om, ev[:, :, 0:64],
                             zrp[:, :, 0:1].broadcast_to(bsh))
        t0 = sbB.tile([P, 4, DH], f32, tag="t0", name=f"t0{uid}")
        nc.gpsimd.tensor_mul(t0, ev[:, :, 194:258],
                             zrp[:, :, 2:3].broadcast_to(bsh))
        t1 = sbB.tile([P, 4, DH], f32, tag="t1", name=f"t1{uid}")
        nc.gpsimd.tensor_mul(t1, ev[:, :, 65:129],
                             zrp[:, :, 3:4].broadcast_to(bsh))
        ov = sbB.tile([P, 4, DH], f32, tag="ov", name=f"ov{uid}")
        nc.gpsimd.tensor_add(t0, ev[:, :, 129:193], t0)
        nc.gpsimd.tensor_add(t0, t0, t1)
        nc.gpsimd.tensor_mul(ov, t0, zrp[:, :, 1:2].broadcast_to(bsh))
        nc.sync.dma_start(
            outm_d[ic * 512:(ic + 1) * 512, h * 64:(h + 1) * 64]
            .rearrange("(ib p) d -> p ib d", p=P),
            om,
        )
        nc.sync.dma_start(
            outv_d[ic * 512:(ic + 1) * 512, h * 64:(h + 1) * 64]
            .rearrange("(ib p) d -> p ib d", p=P),
            ov,
        )

    def scores_unit(h, ic, t, KVT, av_work):
        """Score matmuls + elementwise for one (h, ic) unit; between score
        steps, drain the previous unit's deferred AV matmul emitters so the
        PE never sits behind an exp-paced PSUM-bank wait.

        tj's are processed in PAIRS sharing one [P, 2, 512] scm PSUM tile so
        the serial ACT exp chain (queue depth 0) runs half as many ops; the
        odd plane's tail columns beyond its causal width hold stale values
        whose exp/u/pw garbage stays in columns no AV matmul ever reads.
        """
        ntj = 4 * (ic + 1)
        uid = f"{h}_{ic}"
        nstep = ntj // 2
        per_step = (len(av_work) + nstep - 1) // nstep if av_work else 0
        pv = []  # (pv_tile, plane, u, w_, pw, i0)
        for pr in range(nstep):
            tjs = (2 * pr, 2 * pr + 1)
            i0s = [max(ic * 512, tj * P) for tj in tjs]
            Ws = [(ic + 1) * 512 - i0 for i0 in i0s]
            We = Ws[0]
            scm2 = psumS.tile([P, 2, 512], f32, tag="scm2", bufs=1,
                              name=f"scm{uid}_{pr}")
            scvs = []
            for q, tj in enumerate(tjs):
                jsl = slice(tj * P, (tj + 1) * P)
                isl = slice(i0s[q], i0s[q] + Ws[q])
                scv = psumS.tile([P, 512], f32, tag="scv", bufs=3,
                                 name=f"scv{uid}_{tj}")
                diag = i0s[q] == tj * P
                nc.tensor.matmul(
                    scm2[:, q, 0:Ws[q]], KVT[:, 0:2, jsl],
                    Q4[:, 0:2, t, isl],
                    start=True, stop=not diag, perf_mode=DR,
                    skip_group_check=True,
                )
                if diag:
                    nc.tensor.matmul(
                        scm2[:, q, 0:P], identb, tneg,
                        start=False, stop=True, skip_group_check=True,
                    )
                nc.tensor.matmul(
                    scv[:, 0:Ws[q]], KVT[:, 2:4, jsl], Q4[:, 1:3, t, isl],
                    start=True, stop=True, perf_mode=DR,
                )
                scvs.append(scv)
            p2 = sbB.tile([P, 2, 512], bf16, tag="p2", bufs=7,
                          name=f"p2{uid}_{pr}")
            nc.scalar.activation(p2[:, :, 0:We], scm2[:, :, 0:We], AF.Exp,
                                 scale=2.0 ** -3)
            u2 = sbB.tile([P, 2, 512], bf16, tag="u2", bufs=7,
                          name=f"u2{uid}_{pr}")
            nc.vector.tensor_mul(u2[:, :, 0:We], p2[:, :, 0:We],
                                 p2[:, :, 0:We])
            # w' = u * scv straight from PSUM (no sv evacuation); DVE only:
            # the Pool engine cannot address PSUM on hardware
            w2 = sbB.tile([P, 2, 512], bf16, tag="w2", bufs=7,
                          name=f"w2{uid}_{pr}")
            for q, tj in enumerate(tjs):
                nc.vector.tensor_mul(w2[:, q, 0:Ws[q]], u2[:, q, 0:Ws[q]],
                                     scvs[q][:, 0:Ws[q]])
            pw2 = sbB.tile([P, 2, 512], bf16, tag="pw2", bufs=7,
                           name=f"pw2{uid}_{pr}")
            nc.vector.tensor_mul(pw2[:, :, 0:We], p2[:, :, 0:We],
                                 w2[:, :, 0:We])
            for q in range(2):
                pv.append((p2, u2, w2, pw2, q, i0s[q]))
            for _ in range(per_step):
                if av_work:
                    av_work.pop(0)()
        while av_work:
            av_work.pop(0)()
        return pv

    # software pipeline: the AV matmuls of unit u are interleaved between
    # the score steps of unit u+1; u's combine is emitted right after.
    wflip = [0]
    prepped = {0: prep_pair(0)}
    pending = None  # (h, ic, avt)
    av_work = []
    for t in range(4):  # head pairs
        KVTs, Rs = prepped.pop(t)
        for r in range(2):
            if r == 1 and t < 3:
                prepped[t + 1] = prep_pair(t + 1)
            h = 2 * t + r
            # last head runs ic=1 first so the un-overlapped tail is the
            # short 4-tile unit
            ics = (1, 0) if (t, r) == (3, 1) else (0, 1)
            for ic in ics:
                pv = scores_unit(h, ic, t, KVTs[r], av_work)
                if pending is not None:
                    combine(*pending)
                avt, av_work = av_tj_emitters(h, ic, Rs[r], pv)
                pending = (h, ic, avt)
    while av_work:
        av_work.pop(0)()
    combine_direct(*pending)

    for pool in (cpool, psumS, psumB, sbB, pairp, qpool):
        pool.release()


def build():
    global _BUILT
    if _BUILT is not None:
        return _BUILT
    import concourse.tile as tile
    from concourse import bacc

    nc = bacc.Bacc("TRN2", target_bir_lowering=False, debug=False)
    with tile.TileContext(nc) as tc:
        _emit(nc, tc)
    nc.compile()
    _BUILT = nc
    return nc


def _t_tiles(a):
    """[S, D'] -> [P, D'/128-tiles, S] transposed tile layout."""
    dp = a.shape[1] // P
    return np.ascontiguousarray(a.T.reshape(dp, P, -1).transpose(1, 0, 2))


def make_in_maps(inputs):
    import ml_dtypes

    f8 = ml_dtypes.float8_e4m3
    x = np.asarray(inputs["x"], dtype=np.float32)
    var_x = np.asarray(inputs["var_x"], dtype=np.float32)
    k = np.asarray(inputs["k"], dtype=np.float32)
    var_k = np.asarray(inputs["var_k"], dtype=np.float32)
    v = np.asarray(inputs["v"], dtype=np.float32)
    var_v = np.asarray(inputs["var_v"], dtype=np.float32)
    wq_mu = np.asarray(inputs["Wq_mu"], dtype=np.float32)
    wq_var = np.asarray(inputs["Wq_var"], dtype=np.float32)

    in_maps = []
    for c in range(8):
        b, g = c // 2, c % 2
        gs = slice(g * GCOL, (g + 1) * GCOL)
        hs = slice(g * NHC, (g + 1) * NHC)

        xT = _t_tiles(x[b]).astype(f8)
        varxT = _t_tiles(var_x[b] * 2.0 ** 4).astype(f8)

        wmu = wq_mu[gs]                       # [GCOL, D]
        wvar = wq_var[gs]
        w1 = wvar + wmu ** 2

        def wtiles(w, sc):
            # [GCOL, D] -> W^T tiles [P(d-in-tile), KD, CT, P(col)]
            wt = (w.T * sc).reshape(KD, P, CT, P)
            return np.ascontiguousarray(wt.transpose(1, 0, 2, 3)).astype(f8)

        wqmuT = wtiles(wmu, 2.0 ** 5)
        w1T = wtiles(w1, 2.0 ** 7)
        # 2^11 (not 2^13): X2t holds x^2 (not x^2/4, it is squared on-chip
        # from the fp8 x), keep both var accumulation terms at 2^11
        wqvarT = wtiles(wvar, 2.0 ** 11)

        # per-head fp8 planes (kT*2^-2 | 0 | kv2*2^-2 | var_k*2^3): head 2t+r
        # occupies rows [r*64, r*64+64) (matching its q partitions)
        kg = k[b, hs]                          # [NHC, S, DH]
        vkg = var_k[b, hs]
        kv2 = vkg + kg ** 2
        kvT = np.zeros((NHC, P, 4, S), dtype=np.float32)
        for hh in range(NHC):
            rows = slice((hh % 2) * DH, (hh % 2) * DH + DH)
            kvT[hh, rows, 0] = kg[hh].T * 2.0 ** -2
            kvT[hh, rows, 2] = kv2[hh].T * 2.0 ** -2
            # 2^1 (not 2^3): qT2 holds q^2 (not q^2/4), keep both scv
            # accumulation terms at the same 2x scale
            kvT[hh, rows, 3] = vkg[hh].T * 2.0 ** 1
        kvT = kvT.astype(f8)

        # R tiles [NHC, P, KD, RC]: [v | 1 | var_v | vv2 | vv2*2^-11 | 2^-11]
        vg = v[b, hs].reshape(NHC, KD, P, DH).transpose(0, 2, 1, 3)
        vvg = var_v[b, hs].reshape(NHC, KD, P, DH).transpose(0, 2, 1, 3)
        Rh = np.empty((NHC, P, KD, RC), dtype=np.float32)
        Rh[..., 0:DH] = vg
        Rh[..., 64:65] = 1.0
        Rh[..., 65:129] = vvg + vg ** 2      # vv2 (A1 stationary)
        Rh[..., 129:193] = vvg               # var_v (X/A3 stationary)
        Rh[..., 193:257] = (vvg + vg ** 2) * 2.0 ** -11
        Rh[..., 257:258] = 2.0 ** -11
        Rh = Rh.astype(ml_dtypes.bfloat16)

        in_maps.append({
            "xT8": xT,
            "varxT8": varxT,
            "wqmuT8": wqmuT,
            "w1T8": w1T,
            "wqvarT8": wqvarT,
            "kvT8": kvT,
            "Rh": Rh,
        })
    return in_maps


def assemble(results):
    out_mean = np.empty((B, S, D), np.float32)
    out_var = np.empty((B, S, D), np.float32)
    for c, r in enumerate(results):
        b, g = c // 2, c % 2
        out_mean[b, :, g * GCOL:(g + 1) * GCOL] = r["out_mean"]
        out_var[b, :, g * GCOL:(g + 1) * GCOL] = r["out_var"]
    return out_mean, out_var


def kernel(**inputs):
    from concourse.bass_utils import run_bass_kernel_spmd

    nc = build()
    in_maps = make_in_maps(inputs)
    res = run_bass_kernel_spmd(nc, in_maps, core_ids=list(range(8)))
    out_mean, out_var = assemble(res.results)
    # residual x is added on the host (saves a DMA + an add pass per unit)
    out_mean += np.asarray(inputs["x"], dtype=np.float32)
    return out_mean, out_var
